# revision 35
# baseline (speedup 1.0000x reference)
"""BERT encoder (12 layers, B=8, S=512, H=768, NH=12, FF=3072) on 8 TRN2
NeuronCores. Data-parallel over batch: each core runs the full 12-layer
encoder on one batch element; no collectives.

On-chip strategy (per core, per layer):
- Activations live feature-major in SBUF as float32r (fp32 with 12-bit
  mantissa, full PE rate): X^T [H=768 (6 x 128-partition chunks), S=512
  free]. Projection matmuls use the DRAM-natural weight layout as the
  stationary operand; no transposes anywhere on chip.
- Weights are float16 in DRAM and SBUF (halves HBM traffic and LDWEIGHTS
  time); matmuls mix f16 stationary x f32r moving, fp32 PSUM accumulate.
- Attention: scores^T[k,q] per head via K=64 matmuls on 64-partition
  slices; exp (with 1/sqrt(64) scale) fused in one ACT op into f32r
  probs; ctx via lhsT = V_tok2 slice with an appended ones-column so the
  softmax denominator lands in the same PSUM tile; per-q normalization
  via DVE reciprocal_approx_fast + GpSimd partition_broadcast + one DVE
  multiply (no PE broadcast matmuls, no 4us iterative reciprocals).
- LayerNorm (feature-major = partition direction): sums/sums-of-squares
  via ones-column matmuls on f32r tiles (bias+residual written directly
  as f32r by one scalar_tensor_tensor; square on ACT); rstd via ACT
  Rsqrt; mean/rstd rows broadcast across partitions on GpSimd; normalize
  is 2 in-place DVE tensor_tensor ops + 1 tensor_scalar affine per chunk.
- FFN interleaved per 128-wide inter chunk: 6 FFN1 matmuls -> fused
  bias+gelu (exact erf gelu on ACT) -> 6 FFN2 accumulating matmuls, so
  the 6 MB intermediate never materializes.
"""
import os
import sys
import numpy as np

sys.path.insert(0, '/opt/trn_rl_repo')

L, B, S, H, NH, DH, FF = 12, 8, 512, 768, 12, 64, 3072
L = int(os.environ.get("KERNEL_LAYERS", L))
HC = H // 128      # 6 hidden chunks
FC = FF // 128     # 24 ff chunks
SC = S // 128      # 4 token chunks
EPS = 1e-12

_CACHE = {}


def _build_program():
    import concourse.tile as tile
    from concourse import bacc, mybir

    F32 = mybir.dt.float32
    F32R = mybir.dt.float32r
    F16 = mybir.dt.float16
    AFT = mybir.ActivationFunctionType
    ALU = mybir.AluOpType

    nc = bacc.Bacc("TRN2", target_bir_lowering=False, debug=False)

    # Steer the ACT table chooser to the combined ln+exp table so the
    # per-layer Exp/Ln mix is served by ONE table (masked tables keep their
    # positional act_func_set_id; an empty set is just never selected).
    import types
    import bass_rust as _bass_rust
    from concourse.hw_specs import get_activation_tables

    def _patched_atl(self):
        has_act = any(isinstance(i, mybir.InstActivation)
                      for b in self.main_func.blocks for i in b.instructions)
        if not has_act:
            return
        tables = []
        for name, funcs in get_activation_tables(self.m.arch).items():
            if name in ("exp_and_others", "natural_log", "exp_and_friends"):
                funcs = set()
            tables.append((name, funcs))
        _bass_rust.insert_act_table_loads(self, tables)

    nc.insert_act_table_loads = types.MethodType(_patched_atl, nc)

    hsT = nc.dram_tensor("hsT", [H, S], F16, kind="ExternalInput").ap()
    ident = nc.dram_tensor("ident", [128, 128], F16, kind="ExternalInput").ap()
    Wq = nc.dram_tensor("Wq", [L, H, H], F16, kind="ExternalInput").ap()
    Wk = nc.dram_tensor("Wk", [L, H, H], F16, kind="ExternalInput").ap()
    Wv = nc.dram_tensor("Wv", [L, H, H], F16, kind="ExternalInput").ap()
    Wo = nc.dram_tensor("Wo", [L, H, H], F16, kind="ExternalInput").ap()
    Wi = nc.dram_tensor("Wi", [L, H, FF], F16, kind="ExternalInput").ap()
    Wo2 = nc.dram_tensor("Wo2", [L, FF, H], F16, kind="ExternalInput").ap()
    # packed per-layer 768-vecs:
    # bq,bk,bo_eff,g1,b1,g2,b2,bo2,Sq,Sk -> [L,128,10*HC]
    vecs = nc.dram_tensor("vecs", [L, 128, 10 * HC], F32,
                          kind="ExternalInput").ap()
    biv = nc.dram_tensor("biv", [L, 128, FC], F32, kind="ExternalInput").ap()
    outT = nc.dram_tensor("outT", [H, S], F16, kind="ExternalOutput").ap()

    no_gpsimd = not bool(os.environ.get("KERNEL_GPSIMD"))
    slow_recip = bool(os.environ.get("KERNEL_SLOW_RECIP"))
    trace_sim = bool(os.environ.get("KERNEL_TRACE_SIM"))
    with tile.TileContext(nc, trace_sim=trace_sim) as tc, \
            nc.allow_low_precision(reason="f32r/f16 matmul pipeline"):
        with (
            tc.tile_pool(name="persist", bufs=1) as pp,
            tc.tile_pool(name="actf", bufs=2) as pf,     # f32r [128,3072]
            tc.tile_pool(name="actr", bufs=2) as pr,     # f32r [128,3072]
            tc.tile_pool(name="w768", bufs=36) as pw,    # f16 weight chunks
            tc.tile_pool(name="small", bufs=3) as psm,
            tc.tile_pool(name="probs", bufs=4) as ppr,
            tc.tile_pool(name="inter", bufs=2) as pit,
            tc.tile_pool(name="bias", bufs=2) as pb,
            tc.tile_pool(name="stat", bufs=3) as pst,
            tc.tile_pool(name="psum", bufs=2, space="PSUM") as ps,
        ):
            ones32 = pp.tile([128, 128], F32, tag="ones32", name="ones32")
            nc.vector.memset(ones32[:], 1.0)
            zeros32 = pp.tile([128, 64], F32, tag="zeros32", name="zeros32")
            nc.vector.memset(zeros32[:], 0.0)
            onec = pp.tile([128, 1], F16, tag="onec", name="onec")
            nc.vector.tensor_copy(onec[:], ones32[:, 0:1])
            ones_h = pp.tile([128, 128], F16, tag="onesh", name="ones_h")
            nc.vector.tensor_copy(ones_h[:], ones32[:, :])

            def pbcast(out_sb, row_f16, psum_pool, ptag, pbufs, nm, drow=0):
                """[1,512] f16 row -> [128,512] f16 bcast tile (SBUF)."""
                if no_gpsimd:
                    bp = psum_pool.tile([128, 512], F32, tag=ptag, bufs=pbufs,
                                        name=f"bp_{nm}")
                    nc.tensor.matmul(bp[:], ones_h[drow:drow + 1, :],
                                     row_f16, start=True, stop=True)
                    nc.vector.tensor_copy(out_sb, bp[:])
                else:
                    nc.gpsimd.partition_broadcast(out_sb, row_f16)

            eps_t = pp.tile([1, 1], F32, tag="eps", name="eps_t")
            nc.vector.memset(eps_t[:], EPS)
            id_t = pp.tile([128, 128], F16, tag="ident", name="id_t")
            nc.sync.dma_start(id_t[:], ident)
            dumt = pp.tile([1, 2], F32, tag="dumt", name="dumt")
            nc.vector.memset(dumt[:], 1.0)

            xT = pp.tile([128, HC * 512], F16, tag="xT", name="xT")
            nc.sync.dma_start(xT[:].rearrange("p (c s) -> p c s", c=HC),
                              hsT.rearrange("(c p) s -> p c s", p=128))

            qT = pp.tile([128, HC * 512], F16, tag="qT", name="qT")
            kT = pp.tile([128, HC * 512], F16, tag="kT", name="kT")
            ctxT = pp.tile([128, HC * 512], F16, tag="ctxT", name="ctxT")
            # v_tok2: [s-chunk][head][128 cols]; even head [v(64)|1|z63],
            # odd head [1|z63|v(64)]
            vt = pp.tile([128, SC * NH * 128], F16, tag="vt", name="vt")
            vt4 = vt[:].rearrange("p (sc h c) -> p sc h c", sc=SC, h=NH)
            nc.vector.tensor_copy(
                vt4[:, :, 0::2, 64:65],
                ones32[:, None, None, 0:1].broadcast_to([128, SC, 6, 1]))
            nc.vector.tensor_copy(
                vt4[:, :, 0::2, 65:128],
                zeros32[:, None, None, 0:63].broadcast_to([128, SC, 6, 63]))
            nc.vector.tensor_copy(
                vt4[:, :, 1::2, 0:1],
                ones32[:, None, None, 0:1].broadcast_to([128, SC, 6, 1]))
            nc.vector.tensor_copy(
                vt4[:, :, 1::2, 1:64],
                zeros32[:, None, None, 0:63].broadcast_to([128, SC, 6, 63]))

            def mmslice(t, c):
                return t[:, c * 512:(c + 1) * 512]

            def ln_finish(x_master, sum_row, sq_row, g_j, b_j, vec_t, tag,
                          out_r, bc_pool, bc_tag, bc_bufs, scratch=None):
                """Feature-dim LN: short stats chain -> rstd via Exp(-.5 Ln)
                -> PE broadcast -> 3 DVE passes per chunk into out_r (f16).
                The mean-add passes are emitted first (they only need mb, so
                DVE runs them while the rstd ACT chain finishes). If
                `scratch` is given the passes write there, leaving x_master
                raw. Returns (mb, rb)."""
                def vslot(j, c):
                    return vec_t[:, j * HC + c: j * HC + c + 1]

                mneg_h = pst.tile([1, 512], F16, tag="stat16", bufs=4,
                                  name=f"mnegh_{tag}")
                nc.vector.tensor_scalar(mneg_h[:], sum_row, -1.0 / H, None,
                                        ALU.mult)
                m2 = pst.tile([1, 512], F32, tag="stat32", bufs=6,
                              name=f"m2_{tag}")
                nc.scalar.activation(m2[:], sum_row, AFT.Square,
                                     scale=1.0 / H)
                var = pst.tile([1, 512], F32, tag="stat32", bufs=6,
                               name=f"var_{tag}")
                nc.vector.scalar_tensor_tensor(var[:], sq_row, 1.0 / H,
                                               m2[:], ALU.mult, ALU.subtract)
                lnv = pst.tile([1, 512], F32, tag="stat32", bufs=6,
                               name=f"lnv_{tag}")
                nc.scalar.activation(lnv[:], var[:], AFT.Ln,
                                     bias=eps_t[0:1, :])
                rstd_h = pst.tile([1, 512], F16, tag="stat16", bufs=4,
                                  name=f"rstdh_{tag}")
                nc.scalar.activation(rstd_h[:], lnv[:], AFT.Exp, scale=-0.5)
                mb = psm.tile([128, 512], F16, tag="bcast", bufs=5,
                              name=f"mb_{tag}")
                pbcast(mb[:], mneg_h[0:1, :], bc_pool, bc_tag, bc_bufs,
                       f"mb_{tag}")
                tgt = scratch if scratch is not None else x_master
                for c in range(HC):
                    nc.vector.tensor_tensor(mmslice(tgt, c),
                                            mmslice(x_master, c), mb[:],
                                            ALU.add)

                def tail():
                    """rb broadcast (one PE matmul) + mult/affine passes.
                    Deferred by LN2 into the next layer's pipeline so PE has
                    chains to run while the rstd ACT chain finishes."""
                    rb = psm.tile([128, 512], F16, tag="bcast", bufs=5,
                                  name=f"rb_{tag}")
                    pbcast(rb[:], rstd_h[0:1, :], bc_pool, bc_tag, bc_bufs,
                           f"rb_{tag}")
                    for c in range(HC):
                        nc.vector.tensor_tensor(mmslice(tgt, c),
                                                mmslice(tgt, c), rb[:],
                                                ALU.mult)
                        nc.vector.tensor_scalar(mmslice(out_r, c),
                                                mmslice(tgt, c),
                                                vslot(g_j, c), vslot(b_j, c),
                                                ALU.mult, ALU.add)
                    return rb

                return mb, tail

            fpx = xT          # raw pre-LN2 master feeding Q/K (input at li=0)
            rb_prev = None    # LN2 rstd broadcast from the previous layer
            tneg_prev = None  # broadcast of -mu*rstd from the previous layer
            pending_ln2 = None  # deferred LN2 tail (rb bcast + mult/affine)
            for li in range(L):
                vec_t = pb.tile([128, 10 * HC], F32, tag="vec",
                                name=f"vec_{li}")
                nc.sync.dma_start(vec_t[:], vecs[li])
                bi_t = pb.tile([128, FC], F32, tag="biv", name=f"biv_{li}")
                nc.sync.dma_start(bi_t[:], biv[li])

                def vslot(j, c):
                    return vec_t[:, j * HC + c: j * HC + c + 1]

                wq_t = [pw.tile([128, H], F16, tag="w768",
                                name=f"wq_{li}_{c}") for c in range(HC)]
                for c in range(HC):
                    nc.sync.dma_start(wq_t[c][:], Wq[li, c * 128:(c + 1) * 128, :])
                wk_t = [pw.tile([128, H], F16, tag="w768",
                                name=f"wk_{li}_{c}") for c in range(HC)]
                for c in range(HC):
                    nc.sync.dma_start(wk_t[c][:], Wk[li, c * 128:(c + 1) * 128, :])
                wv_t = [pw.tile([128, H], F16, tag="w768",
                                name=f"wv_{li}_{c}") for c in range(HC)]
                for c in range(HC):
                    nc.sync.dma_start(wv_t[c][:], Wv[li, c * 128:(c + 1) * 128, :])

                wo_t = [pw.tile([128, H], F16, tag="w768",
                                name=f"wo_{li}_{c}") for c in range(HC)]
                for c in range(HC):
                    nc.sync.dma_start(wo_t[c][:], Wo[li, c * 128:(c + 1) * 128, :])

                # ---- fused QKV + attention, software-pipelined over head
                # pairs. PE emission interleaves scores matmuls between the
                # Q/K/V accumulation chains so TensorE never waits on the
                # ACT exp stream (HAM stays warm); ctx runs one slot behind
                # scores, epilogue one behind ctx. PSUM budget: qkv 2 +
                # scores 2 + ctx 2 (+ global ps 2) = 8 banks.
                att_pool_cm = tc.tile_pool(name=f"att{li}", bufs=1,
                                           space="PSUM")
                pat = att_pool_cm.__enter__()

                pr_eo = {}
                ctx_eo = {}

                def emit_qkchain(m, w_t, dst, bias_j):
                    """Q/K chain on the RAW pre-LN2 master (fpx), evacuated
                    via ACT; the per-token LN fixup runs on DVE afterwards,
                    off the PE critical path (emit_qkfix)."""
                    q_ps = pat.tile([128, 512], F32, tag="qkp", bufs=2,
                                    name=f"{'qps' if bias_j == 0 else 'kps'}"
                                         f"_{li}_{m}")
                    for c in range(HC):
                        nc.tensor.matmul(q_ps[:],
                                         w_t[c][:, m * 128:(m + 1) * 128],
                                         mmslice(fpx, c), start=(c == 0),
                                         stop=(c == HC - 1),
                                         skip_group_check=True)
                    if li == 0:
                        nc.scalar.activation(mmslice(dst, m), q_ps[:],
                                             AFT.Identity, bias=vslot(bias_j, m))
                    else:
                        nc.scalar.activation(mmslice(dst, m), q_ps[:],
                                             AFT.Identity)

                def emit_qkfix(m, dst, bias_j, s_j):
                    """q = rstd*qraw - (mu*rstd)*S + b' (per-token LN2 fold)."""
                    if li == 0:
                        return
                    nc.vector.tensor_tensor(mmslice(dst, m),
                                            mmslice(dst, m), rb_prev[:],
                                            ALU.mult)
                    nc.vector.scalar_tensor_tensor(
                        mmslice(dst, m), tneg_prev[:], vslot(s_j, m),
                        mmslice(dst, m), ALU.mult, ALU.add)
                    nc.vector.tensor_scalar(mmslice(dst, m),
                                            mmslice(dst, m),
                                            vslot(bias_j, m), None,
                                            ALU.add)

                def emit_qchain(m):
                    emit_qkchain(m, wq_t, qT, 0)

                def emit_kchain(m):
                    emit_qkchain(m, wk_t, kT, 1)

                def emit_vgroup(sc, half):
                    v_ps = pat.tile([128, 384], F32, tag="qkp", bufs=2,
                                    name=f"vps_{li}_{sc}_{half}")
                    for c in range(HC):
                        nc.tensor.matmul(
                            v_ps[:],
                            xT[:, c * 512 + sc * 128:
                               c * 512 + (sc + 1) * 128],
                            wv_t[c][:, half * 384:(half + 1) * 384],
                            start=(c == 0), stop=(c == HC - 1),
                            skip_group_check=True)
                    v3 = v_ps[:].rearrange("p (h x c) -> p h x c", h=3, x=2)
                    nc.vector.tensor_copy(
                        vt4[:, sc, half * 6 + 0:half * 6 + 6:2, 0:64],
                        v3[:, :, 0, :])
                    nc.vector.tensor_copy(
                        vt4[:, sc, half * 6 + 1:half * 6 + 6:2, 64:128],
                        v3[:, :, 1, :])

                def emit_scores_kc(hp, kc):
                    c = hp
                    if kc == 0:
                        pr_eo[hp] = [ppr.tile([128, SC * 512], F16,
                                              tag="probs",
                                              name=f"probs_{li}_{2*hp+r}")
                                     for r in range(2)]
                    st_eo = [pat.tile([128, 512], F32, tag="satt", bufs=2,
                                      name=f"sps_{li}_{hp}_{r}_{kc}")
                             for r in range(2)]
                    # r0/r1 matmuls interleaved: disjoint PE row groups
                    # (rows 0-63 vs 64-127) execute concurrently
                    for r in range(2):
                        o = r * 64
                        nc.tensor.matmul(
                            st_eo[r][:],
                            kT[o:o + 64, c * 512 + kc * 128:
                               c * 512 + (kc + 1) * 128],
                            qT[o:o + 64, c * 512:(c + 1) * 512],
                            start=True, stop=True, skip_group_check=True)
                    for r in range(2):
                        nc.scalar.activation(
                            pr_eo[hp][r][:, kc * 512:(kc + 1) * 512],
                            st_eo[r][:], AFT.Exp,
                            scale=float(1.0 / np.sqrt(DH)))

                def emit_ctx_kc(hp, kcs):
                    for r in range(2):
                        h = 2 * hp + r
                        if (hp, r) not in ctx_eo:
                            ctx_eo[(hp, r)] = pat.tile(
                                [128, 512], F32, tag="ctx", bufs=2,
                                name=f"cps_{li}_{h}")
                        ctx_ps = ctx_eo[(hp, r)]
                        for kc in kcs:
                            lhs = (vt4[:, kc, h, 0:65] if r == 0
                                   else vt4[:, kc, h, 0:128])
                            nc.tensor.matmul(
                                ctx_ps[0:(65 if r == 0 else 128), :], lhs,
                                pr_eo[hp][r][:, kc * 512:(kc + 1) * 512],
                                start=(kc == 0), stop=(kc == SC - 1),
                                skip_group_check=True)

                def emit_epi(hp):
                    c = hp
                    c0 = ctx_eo.pop((hp, 0))
                    c1 = ctx_eo.pop((hp, 1))
                    del pr_eo[hp]
                    # softmax denominators sit in ctx PSUM rows (64 for even
                    # head, 0 for odd). 1/den via one fast-recip DVE op each
                    # (custom-DVE ops need base partition 0, so slice from
                    # row 0); PE K=1 matmul broadcasts the f32 rec row (as
                    # f32r moving) across partitions; one multiply per head
                    # normalizes ctx into ctxT.
                    from concourse.dve_ops import (RECIP_APPROX_FAST_CONSTS,
                                                   RECIPROCAL_APPROX_FAST)
                    rc = RECIP_APPROX_FAST_CONSTS
                    rec0 = psm.tile([128, 512], F16, tag="rec", bufs=2,
                                    name=f"reca_{li}_{hp}")
                    nc.vector._custom_dve(
                        RECIPROCAL_APPROX_FAST,
                        out=rec0[0:65, :], in0=c0[0:65, :],
                        s0=rc["s0"], s1=rc["s1"], imm2=rc["imm2"])
                    rec1 = psm.tile([128, 512], F16, tag="rec", bufs=2,
                                    name=f"recb_{li}_{hp}")
                    nc.vector._custom_dve(
                        RECIPROCAL_APPROX_FAST,
                        out=rec1[0:1, :], in0=c1[0:1, :],
                        s0=rc["s0"], s1=rc["s1"], imm2=rc["imm2"])
                    bb = psm.tile([128, 512], F16, tag="bsb", bufs=3,
                                  name=f"bb_{li}_{hp}")
                    bp0 = ps.tile([128, 512], F32, tag="ps", bufs=2,
                                  name=f"bpa_{li}_{hp}")
                    nc.tensor.matmul(bp0[:], ones_h[64:65, :],
                                     rec0[64:65, :], start=True, stop=True)
                    nc.vector.tensor_copy(bb[0:64, :], bp0[0:64, :])
                    bp1 = ps.tile([128, 512], F32, tag="ps", bufs=2,
                                  name=f"bpb_{li}_{hp}")
                    nc.tensor.matmul(bp1[:], ones_h[0:1, :],
                                     rec1[0:1, :], start=True, stop=True)
                    nc.vector.tensor_copy(bb[64:128, :], bp1[64:128, :])
                    nc.vector.tensor_tensor(
                        ctxT[0:64, c * 512:(c + 1) * 512],
                        c0[0:64, :], bb[0:64, :], ALU.mult)
                    nc.vector.tensor_tensor(
                        ctxT[64:128, c * 512:(c + 1) * 512],
                        c1[64:128, :], bb[64:128, :], ALU.mult)

                # pipeline: scores(hp) in slot hp+1 interleaved between the
                # Q/K/V chains (each chain gives ACT time to drain the
                # previous kc's exps); ctx(hp) in slot hp+2. V groups read
                # the NORMALIZED xT (ready a few us into the layer), so they
                # sit in slots 2-3, just before the ctx stages that consume
                # them (ctx(hp<3) needs half 0, ctx(hp>=3) half 1).
                vplan = {2: [(0, 0), (1, 0), (2, 0), (3, 0)],
                         3: [(0, 1), (1, 1), (2, 1), (3, 1)]}
                for slot in range(8):
                    hp_s = slot - 1
                    hp_c = slot - 2
                    vg = vplan.get(slot, [])
                    if 0 <= hp_s < 6:
                        emit_scores_kc(hp_s, 0)
                    if slot < HC:
                        emit_qchain(slot)
                    if 0 <= hp_s < 6:
                        emit_scores_kc(hp_s, 1)
                    if slot < HC:
                        emit_kchain(slot)
                    if slot == 0 and pending_ln2 is not None:
                        # previous layer's deferred LN2 tail: its rb
                        # broadcast matmul now sits behind this slot's 12
                        # chain matmuls, so PE stays busy while the rstd
                        # ACT chain completes.
                        mb2p, tail2p = pending_ln2
                        rb_prev = tail2p()
                        tneg_prev = psm.tile([128, 512], F16, tag="tneg",
                                             bufs=2, name=f"tneg_{li}")
                        nc.vector.tensor_tensor(tneg_prev[:], mb2p[:],
                                                rb_prev[:], ALU.mult)
                        pending_ln2 = None
                    if slot < HC:
                        emit_qkfix(slot, qT, 0, 8)
                        emit_qkfix(slot, kT, 1, 9)
                    if 0 <= hp_s < 6:
                        emit_scores_kc(hp_s, 2)
                    for v in vg[:2]:
                        emit_vgroup(*v)
                    if 0 <= hp_s < 6:
                        emit_scores_kc(hp_s, 3)
                    for v in vg[2:]:
                        emit_vgroup(*v)
                    if hp_c >= 0:
                        emit_ctx_kc(hp_c, (0, 1))
                        emit_ctx_kc(hp_c, (2, 3))
                        emit_epi(hp_c)

                att_pool_cm.__exit__(None, None, None)

                # ---- attn output projection + residual + inline LN1 stats --
                ln1_pool_cm = tc.tile_pool(name=f"lnp{li}", bufs=2,
                                           space="PSUM")
                pln = ln1_pool_cm.__enter__()
                ap_ = pf.tile([128, HC * 512], F16, tag="actf",
                              name=f"ap_{li}")
                l1sq = pr.tile([128, HC * 512], F16, tag="actr",
                               name=f"l1sq_{li}")
                sum1 = pln.tile([1, 512], F32, tag="lnp", name=f"sum1_{li}")
                sq1 = pln.tile([1, 512], F32, tag="lnp", name=f"sq1_{li}")
                for m in range(HC):
                    a_ps = ps.tile([128, 512], F32, tag="ps",
                                   name=f"aps_{li}_{m}")
                    # residual seeded via identity matmul; evacuation is a
                    # pure ACT Identity+bias (keeps DVE off the tail)
                    nc.tensor.matmul(a_ps[:], id_t[:], mmslice(xT, m),
                                     start=True, stop=False,
                                     skip_group_check=True)
                    for c in range(HC):
                        nc.tensor.matmul(a_ps[:],
                                         wo_t[c][:, m * 128:(m + 1) * 128],
                                         mmslice(ctxT, c), start=False,
                                         stop=(c == HC - 1),
                                         skip_group_check=True)
                    nc.scalar.activation(mmslice(ap_, m), a_ps[:],
                                         AFT.Identity, bias=vslot(2, m))
                    nc.vector.tensor_tensor(mmslice(l1sq, m),
                                            mmslice(ap_, m), mmslice(ap_, m),
                                            ALU.mult)
                    nc.tensor.matmul(sum1[:], onec[:, :], mmslice(ap_, m),
                                     start=(m == 0), stop=(m == HC - 1),
                                     skip_group_check=True)
                    nc.tensor.matmul(sq1[:], onec[:, :], mmslice(l1sq, m),
                                     start=(m == 0), stop=(m == HC - 1),
                                     skip_group_check=True)

                lo_r = pr.tile([128, HC * 512], F16, tag="actr",
                               name=f"lor_{li}")
                _, ln1_tail = ln_finish(ap_, sum1[:], sq1[:], 3, 4, vec_t,
                                        f"l1_{li}", lo_r, pln, "lnp", 2)
                ln1_tail()
                ln1_pool_cm.__exit__(None, None, None)
                # prefetch the gelu ACT table before the first real gelu
                nc.scalar.activation(dumt[0:1, 1:2], dumt[0:1, 0:1],
                                     AFT.Gelu)

                # ---- FFN interleaved (own psum pool, 6 acc banks) ----
                ffn_pool_cm = tc.tile_pool(name=f"ffn{li}", bufs=6,
                                           space="PSUM")
                pacc = ffn_pool_cm.__enter__()
                acc_ps = [pacc.tile([128, 512], F32, tag="ffacc",
                                    name=f"facc_{li}_{m}") for m in range(HC)]
                for m in range(HC):
                    # residual (lo_r) seeded via identity matmul
                    nc.tensor.matmul(acc_ps[m][:], id_t[:], mmslice(lo_r, m),
                                     start=True, stop=False,
                                     skip_group_check=True)
                for g in range(4):
                    wig = [pw.tile([128, H], F16, tag="w768",
                                   name=f"wi_{li}_{g}_{c}") for c in range(HC)]
                    for c in range(HC):
                        nc.sync.dma_start(
                            wig[c][:],
                            Wi[li, c * 128:(c + 1) * 128, g * 768:(g + 1) * 768])
                    for fg in range(HC):
                        f = g * HC + fg
                        f1_ps = ps.tile([128, 512], F32, tag="ps",
                                        name=f"f1_{li}_{f}")
                        for c in range(HC):
                            nc.tensor.matmul(
                                f1_ps[:],
                                wig[c][:, fg * 128:(fg + 1) * 128],
                                mmslice(lo_r, c), start=(c == 0),
                                stop=(c == HC - 1))
                        inter = pit.tile([128, 512], F16, tag="inter",
                                         name=f"it_{li}_{f}")
                        nc.scalar.activation(inter[:], f1_ps[:], AFT.Gelu,
                                             bias=bi_t[:, f:f + 1])
                        wo2_t = pw.tile([128, H], F16, tag="w768",
                                        name=f"wo2_{li}_{f}")
                        nc.sync.dma_start(wo2_t[:],
                                          Wo2[li, f * 128:(f + 1) * 128, :])
                        for m in range(HC):
                            nc.tensor.matmul(
                                acc_ps[m][:], wo2_t[:, m * 128:(m + 1) * 128],
                                inter[:], start=False, stop=(f == FC - 1),
                                skip_group_check=True)
                # prefetch the ln/exp ACT table before LN2's Ln
                nc.scalar.activation(dumt[0:1, 1:2], dumt[0:1, 0:1], AFT.Ln)

                fp_ = pf.tile([128, HC * 512], F16, tag="actf",
                              name=f"fp_{li}")
                l2sq = pr.tile([128, HC * 512], F16, tag="actr",
                               name=f"l2sq_{li}")
                sum2 = ps.tile([1, 512], F32, tag="ps", name=f"sum2_{li}")
                sq2 = ps.tile([1, 512], F32, tag="ps", name=f"sq2_{li}")
                for m in range(HC):
                    nc.scalar.activation(mmslice(fp_, m), acc_ps[m][:],
                                         AFT.Identity, bias=vslot(7, m))
                    nc.vector.tensor_tensor(mmslice(l2sq, m),
                                            mmslice(fp_, m), mmslice(fp_, m),
                                            ALU.mult)
                    nc.tensor.matmul(sum2[:], onec[:, :], mmslice(fp_, m),
                                     start=(m == 0), stop=(m == HC - 1),
                                     skip_group_check=True)
                    nc.tensor.matmul(sq2[:], onec[:, :], mmslice(l2sq, m),
                                     start=(m == 0), stop=(m == HC - 1),
                                     skip_group_check=True)
                pending_ln2 = ln_finish(fp_, sum2[:], sq2[:], 5, 6, vec_t,
                                        f"l2_{li}", xT, ps, "ps", 2,
                                        scratch=l2sq)
                fpx = fp_
                ffn_pool_cm.__exit__(None, None, None)

            # flush the last layer's deferred LN2 tail into xT
            pending_ln2[1]()
            nc.sync.dma_start(outT.rearrange("(c p) s -> p c s", p=128),
                              xT[:].rearrange("p (c s) -> p c s", c=HC))

    nc.compile()
    return nc


def _get_runner():
    if "runner" in _CACHE:
        return _CACHE["runner"]
    import jax
    from jax.sharding import Mesh, PartitionSpec
    from jax.experimental.shard_map import shard_map
    from concourse import mybir
    from concourse.bass2jax import (_bass_exec_p, install_neuronx_cc_hook,
                                    partition_id_tensor)

    install_neuronx_cc_hook()
    nc = _build_program()

    pname = nc.partition_id_tensor.name if nc.partition_id_tensor else None
    in_names, out_names, out_avals, zero_outs = [], [], [], []
    for alloc in nc.m.functions[0].allocations:
        if not isinstance(alloc, mybir.MemoryLocationSet):
            continue
        name = alloc.memorylocations[0].name
        if alloc.kind == "ExternalInput":
            if name == pname:
                continue
            in_names.append(name)
        elif alloc.kind == "ExternalOutput":
            out_names.append(name)
            shape = tuple(alloc.tensor_shape)
            dtype = mybir.dt.np(alloc.dtype)
            out_avals.append(jax.core.ShapedArray(shape, dtype))
            zero_outs.append(np.zeros(shape, dtype))
    n_params = len(in_names)
    n_outs = len(out_avals)
    all_in_names = list(in_names) + list(out_names)
    if pname is not None:
        all_in_names = all_in_names + [pname]

    def _body(*args):
        operands = list(args)
        if pname is not None:
            operands.append(partition_id_tensor())
        outs = _bass_exec_p.bind(
            *operands,
            out_avals=tuple(out_avals),
            in_names=tuple(all_in_names),
            out_names=tuple(out_names),
            lowering_input_output_aliases=(),
            sim_require_finite=False,
            sim_require_nnan=False,
            nc=nc,
        )
        return tuple(outs)

    devices = jax.devices()[:B]
    mesh = Mesh(np.asarray(devices), ("core",))
    in_specs = (PartitionSpec("core"),) * (n_params + n_outs)
    out_specs = (PartitionSpec("core"),) * n_outs
    donate = tuple(range(n_params, n_params + n_outs))
    jitted = jax.jit(
        shard_map(_body, mesh=mesh, in_specs=in_specs, out_specs=out_specs,
                  check_rep=False),
        donate_argnums=donate, keep_unused=True)

    runner = {
        "jit": jitted, "in_names": in_names, "out_names": out_names,
        "zero_outs": zero_outs, "mesh": mesh, "devices": devices,
    }
    _CACHE["runner"] = runner
    return runner


def _prep_core_inputs(inputs):
    if L != 12:  # debug: KERNEL_LAYERS override slices the stacks
        inputs = {k: (v[:L] if k not in ("hidden_states", "attention_mask")
                      else v) for k, v in inputs.items()}
    hs = np.asarray(inputs["hidden_states"], np.float32)
    mask = np.asarray(inputs["attention_mask"], np.float32)
    if np.any(mask):
        raise NotImplementedError(
            "kernel compiled for the zero attention_mask this problem "
            "guarantees (spec fill=zeros); nonzero mask unsupported")
    Wv = np.asarray(inputs["Wv"], np.float16)
    Wo = np.asarray(inputs["Wo"], np.float16)
    Wi = np.asarray(inputs["Wi"], np.float16)
    Wo2 = np.asarray(inputs["Wo2"], np.float16)
    bq = np.asarray(inputs["bq"], np.float64)
    bk = np.asarray(inputs["bk"], np.float64)
    bv = np.asarray(inputs["bv"], np.float32)
    bo = np.asarray(inputs["bo"], np.float32)
    bi = np.asarray(inputs["bi"], np.float32)
    bo2 = np.asarray(inputs["bo2"], np.float32)
    g1 = np.asarray(inputs["ln1_g"], np.float32)
    b1 = np.asarray(inputs["ln1_b"], np.float32)
    g2 = np.asarray(inputs["ln2_g"], np.float32)
    b2 = np.asarray(inputs["ln2_b"], np.float32)

    # fold bv into bo: (ctx + bv) @ Wo + bo == ctx @ Wo + (bo + bv @ Wo)
    bo_eff = (bo.astype(np.float64)
              + np.einsum("lh,lho->lo", bv.astype(np.float64),
                          np.asarray(inputs["Wo"], np.float64))
              ).astype(np.float32)

    # fold the previous layer's LN2 affine (g2, b2) into Q/K:
    #   q = Wq'^T·nrm + bq',  Wq' = g2_prev ⊙rows Wq,  bq' = bq + b2_prev@Wq
    # where nrm = (x_raw − μ)·rstd; the kernel computes Wq'^T·x_raw and
    # fixes up with  rstd·qraw − (μ·rstd)·Sq + bq'  (Sq = colsum(Wq')).
    Wq64 = np.asarray(inputs["Wq"], np.float64).copy()
    Wk64 = np.asarray(inputs["Wk"], np.float64).copy()
    Sq = np.zeros((L, H), np.float64)
    Sk = np.zeros((L, H), np.float64)
    bq_eff = bq.copy()
    bk_eff = bk.copy()
    for li in range(1, L):
        g2p = g2[li - 1].astype(np.float64)
        b2p = b2[li - 1].astype(np.float64)
        bq_eff[li] = bq[li] + b2p @ Wq64[li]
        bk_eff[li] = bk[li] + b2p @ Wk64[li]
        Wq64[li] *= g2p[:, None]
        Wk64[li] *= g2p[:, None]
        Sq[li] = Wq64[li].sum(axis=0)
        Sk[li] = Wk64[li].sum(axis=0)
    Wq = Wq64.astype(np.float16)
    Wk = Wk64.astype(np.float16)
    # Sq/Sk must match the f16 weights actually used by the matmuls
    Sq = Wq.astype(np.float64).sum(axis=1)
    Sk = Wk.astype(np.float64).sum(axis=1)
    Sq[0] = 0.0
    Sk[0] = 0.0

    def pack768(v):  # [L,768] -> [L,128,HC] with [l,p,c] = v[l, c*128+p]
        v = np.asarray(v, np.float32)
        return np.ascontiguousarray(v.reshape(L, HC, 128).transpose(0, 2, 1))

    vecs = np.stack([pack768(v) for v in
                     (bq_eff, bk_eff, bo_eff, g1, b1, g2, b2, bo2, Sq, Sk)],
                    axis=2)
    # [L,128,10,HC] -> [L,128,10*HC]
    vecs = np.ascontiguousarray(vecs.reshape(L, 128, 10 * HC))
    biv = np.ascontiguousarray(bi.reshape(L, FC, 128).transpose(0, 2, 1))

    per_core = {
        "hsT": [np.ascontiguousarray(hs[b].T.astype(np.float16))
                for b in range(B)],
        "ident": [np.eye(128, dtype=np.float16)] * B,
    }
    for name, arr in (("Wq", Wq), ("Wk", Wk), ("Wv", Wv), ("Wo", Wo),
                      ("Wi", Wi), ("Wo2", Wo2), ("vecs", vecs), ("biv", biv)):
        per_core[name] = [arr] * B
    return per_core


def run_on_device(inputs, n_timing_runs=0):
    """Execute; returns (output [B,S,H] fp32, exec_seconds or None)."""
    import jax
    from jax.sharding import NamedSharding, PartitionSpec
    runner = _get_runner()
    per_core = _prep_core_inputs(inputs)
    devices = runner["devices"]
    mesh = runner["mesh"]
    sharding = NamedSharding(mesh, PartitionSpec("core"))

    global_args = []
    for name in runner["in_names"]:
        shards = per_core[name]
        arrs = [jax.device_put(shards[c], devices[c]) for c in range(B)]
        gshape = (B * shards[0].shape[0],) + shards[0].shape[1:]
        global_args.append(
            jax.make_array_from_single_device_arrays(gshape, sharding, arrs))

    def zeros_args():
        outs = []
        for z in runner["zero_outs"]:
            arrs = [jax.device_put(z, devices[c]) for c in range(B)]
            gshape = (B * z.shape[0],) + z.shape[1:]
            outs.append(jax.make_array_from_single_device_arrays(
                gshape, sharding, arrs))
        return outs

    out_arrs = runner["jit"](*global_args, *zeros_args())
    jax.block_until_ready(out_arrs)

    exec_s = None
    if n_timing_runs > 0:
        import time
        times = []
        for _ in range(n_timing_runs):
            zo = zeros_args()
            jax.block_until_ready(zo)
            t0 = time.perf_counter()
            out_arrs = runner["jit"](*global_args, *zo)
            jax.block_until_ready(out_arrs)
            times.append(time.perf_counter() - t0)
        exec_s = min(times)

    outT = np.asarray(out_arrs[0]).astype(np.float32).reshape(B, H, S)
    out = np.ascontiguousarray(outT.transpose(0, 2, 1))
    return out, exec_s


def kernel(**inputs) -> np.ndarray:
    out, _ = run_on_device(inputs, n_timing_runs=0)
    return out



# revision 43
# speedup vs baseline: 1.9400x; 1.9400x over previous
"""BERT encoder (12 layers, B=8, S=512, H=768, NH=12, FF=3072) on 8 TRN2
NeuronCores. Data-parallel over batch: each core runs the full 12-layer
encoder on one batch element; no collectives.

On-chip strategy (per core, per layer):
- Activations live feature-major in SBUF as float32r (fp32 with 12-bit
  mantissa, full PE rate): X^T [H=768 (6 x 128-partition chunks), S=512
  free]. Projection matmuls use the DRAM-natural weight layout as the
  stationary operand; no transposes anywhere on chip.
- Weights are float16 in DRAM and SBUF (halves HBM traffic and LDWEIGHTS
  time); matmuls mix f16 stationary x f32r moving, fp32 PSUM accumulate.
- Attention: scores^T[k,q] per head via K=64 matmuls on 64-partition
  slices; exp (with 1/sqrt(64) scale) fused in one ACT op into f32r
  probs; ctx via lhsT = V_tok2 slice with an appended ones-column so the
  softmax denominator lands in the same PSUM tile; per-q normalization
  via DVE reciprocal_approx_fast + GpSimd partition_broadcast + one DVE
  multiply (no PE broadcast matmuls, no 4us iterative reciprocals).
- LayerNorm (feature-major = partition direction): sums/sums-of-squares
  via ones-column matmuls on f32r tiles (bias+residual written directly
  as f32r by one scalar_tensor_tensor; square on ACT); rstd via ACT
  Rsqrt; mean/rstd rows broadcast across partitions on GpSimd; normalize
  is 2 in-place DVE tensor_tensor ops + 1 tensor_scalar affine per chunk.
- FFN interleaved per 128-wide inter chunk: 6 FFN1 matmuls -> fused
  bias+gelu (exact erf gelu on ACT) -> 6 FFN2 accumulating matmuls, so
  the 6 MB intermediate never materializes.
"""
import os
import sys
import numpy as np

sys.path.insert(0, '/opt/trn_rl_repo')

L, B, S, H, NH, DH, FF = 12, 8, 512, 768, 12, 64, 3072
L = int(os.environ.get("KERNEL_LAYERS", L))
HC = H // 128      # 6 hidden chunks
FC = FF // 128     # 24 ff chunks
SC = S // 128      # 4 token chunks
EPS = 1e-12

_CACHE = {}


def _build_program():
    import concourse.tile as tile
    from concourse import bacc, mybir

    F32 = mybir.dt.float32
    F32R = mybir.dt.float32r
    F16 = mybir.dt.float16
    AFT = mybir.ActivationFunctionType
    ALU = mybir.AluOpType

    nc = bacc.Bacc("TRN2", target_bir_lowering=False, debug=False)

    # Steer the ACT table chooser to the combined ln+exp table so the
    # per-layer Exp/Ln mix is served by ONE table (masked tables keep their
    # positional act_func_set_id; an empty set is just never selected).
    import types
    import bass_rust as _bass_rust
    from concourse.hw_specs import get_activation_tables

    def _patched_atl(self):
        has_act = any(isinstance(i, mybir.InstActivation)
                      for b in self.main_func.blocks for i in b.instructions)
        if not has_act:
            return
        tables = []
        for name, funcs in get_activation_tables(self.m.arch).items():
            if name in ("exp_and_others", "natural_log", "exp_and_friends"):
                funcs = set()
            tables.append((name, funcs))
        _bass_rust.insert_act_table_loads(self, tables)

    nc.insert_act_table_loads = types.MethodType(_patched_atl, nc)

    hsT = nc.dram_tensor("hsT", [H, S], F16, kind="ExternalInput").ap()
    ident = nc.dram_tensor("ident", [128, 128], F16, kind="ExternalInput").ap()
    Wq = nc.dram_tensor("Wq", [L, H, H], F16, kind="ExternalInput").ap()
    Wk = nc.dram_tensor("Wk", [L, H, H], F16, kind="ExternalInput").ap()
    Wv = nc.dram_tensor("Wv", [L, H, H], F16, kind="ExternalInput").ap()
    Wo = nc.dram_tensor("Wo", [L, H, H], F16, kind="ExternalInput").ap()
    Wi = nc.dram_tensor("Wi", [L, H, FF], F16, kind="ExternalInput").ap()
    Wo2 = nc.dram_tensor("Wo2", [L, FF, H], F16, kind="ExternalInput").ap()
    # packed per-layer 768-vecs:
    # bq,bk,bo_eff,g1,b1,g2,b2,bo2,Sq,Sk -> [L,128,10*HC]
    vecs = nc.dram_tensor("vecs", [L, 128, 10 * HC], F32,
                          kind="ExternalInput").ap()
    biv = nc.dram_tensor("biv", [L, 128, FC], F32, kind="ExternalInput").ap()
    s1vt = nc.dram_tensor("s1v", [L, 128, FC], F32, kind="ExternalInput").ap()
    outT = nc.dram_tensor("outT", [H, S], F16, kind="ExternalOutput").ap()

    no_gpsimd = not bool(os.environ.get("KERNEL_GPSIMD"))
    slow_recip = bool(os.environ.get("KERNEL_SLOW_RECIP"))
    trace_sim = bool(os.environ.get("KERNEL_TRACE_SIM"))
    with tile.TileContext(nc, trace_sim=trace_sim) as tc, \
            nc.allow_low_precision(reason="f32r/f16 matmul pipeline"):
        with (
            tc.tile_pool(name="persist", bufs=1) as pp,
            tc.tile_pool(name="actf", bufs=2) as pf,     # f32r [128,3072]
            tc.tile_pool(name="actr", bufs=2) as pr,     # f32r [128,3072]
            tc.tile_pool(name="w768", bufs=36) as pw,    # f16 weight chunks
            tc.tile_pool(name="small", bufs=3) as psm,
            tc.tile_pool(name="probs", bufs=4) as ppr,
            tc.tile_pool(name="inter", bufs=2) as pit,
            tc.tile_pool(name="bias", bufs=2) as pb,
            tc.tile_pool(name="stat", bufs=3) as pst,
            tc.tile_pool(name="psum", bufs=2, space="PSUM") as ps,
        ):
            ones32 = pp.tile([128, 128], F32, tag="ones32", name="ones32")
            nc.vector.memset(ones32[:], 1.0)
            zeros32 = pp.tile([128, 64], F32, tag="zeros32", name="zeros32")
            nc.vector.memset(zeros32[:], 0.0)
            onec = pp.tile([128, 1], F16, tag="onec", name="onec")
            nc.vector.tensor_copy(onec[:], ones32[:, 0:1])
            ones_h = pp.tile([128, 128], F16, tag="onesh", name="ones_h")
            nc.vector.tensor_copy(ones_h[:], ones32[:, :])

            def pbcast(out_sb, row_f16, psum_pool, ptag, pbufs, nm, drow=0):
                """[1,512] f16 row -> [128,512] f16 bcast tile (SBUF)."""
                if no_gpsimd:
                    bp = psum_pool.tile([128, 512], F32, tag=ptag, bufs=pbufs,
                                        name=f"bp_{nm}")
                    nc.tensor.matmul(bp[:], ones_h[drow:drow + 1, :],
                                     row_f16, start=True, stop=True)
                    nc.vector.tensor_copy(out_sb, bp[:])
                else:
                    nc.gpsimd.partition_broadcast(out_sb, row_f16)

            eps_t = pp.tile([1, 1], F32, tag="eps", name="eps_t")
            nc.vector.memset(eps_t[:], EPS)
            id_t = pp.tile([128, 128], F16, tag="ident", name="id_t")
            nc.sync.dma_start(id_t[:], ident)
            dumt = pp.tile([1, 2], F32, tag="dumt", name="dumt")
            nc.vector.memset(dumt[:], 1.0)

            xT = pp.tile([128, HC * 512], F16, tag="xT", name="xT")
            nc.sync.dma_start(xT[:].rearrange("p (c s) -> p c s", c=HC),
                              hsT.rearrange("(c p) s -> p c s", p=128))

            qT = pp.tile([128, HC * 512], F16, tag="qT", name="qT")
            kT = pp.tile([128, HC * 512], F16, tag="kT", name="kT")
            ctxT = pp.tile([128, HC * 512], F16, tag="ctxT", name="ctxT")
            # v_tok2: [s-chunk][head][128 cols]; even head [v(64)|1|z63],
            # odd head [1|z63|v(64)]
            vt = pp.tile([128, SC * NH * 128], F16, tag="vt", name="vt")
            vt4 = vt[:].rearrange("p (sc h c) -> p sc h c", sc=SC, h=NH)
            nc.vector.tensor_copy(
                vt4[:, :, 0::2, 64:65],
                ones32[:, None, None, 0:1].broadcast_to([128, SC, 6, 1]))
            nc.vector.tensor_copy(
                vt4[:, :, 0::2, 65:128],
                zeros32[:, None, None, 0:63].broadcast_to([128, SC, 6, 63]))
            nc.vector.tensor_copy(
                vt4[:, :, 1::2, 0:1],
                ones32[:, None, None, 0:1].broadcast_to([128, SC, 6, 1]))
            nc.vector.tensor_copy(
                vt4[:, :, 1::2, 1:64],
                zeros32[:, None, None, 0:63].broadcast_to([128, SC, 6, 63]))

            def mmslice(t, c):
                return t[:, c * 512:(c + 1) * 512]

            def ln_finish(x_master, sum_row, sq_row, g_j, b_j, vec_t, tag,
                          out_r, bc_pool, bc_tag, bc_bufs, scratch=None):
                """Feature-dim LN: short stats chain -> rstd via Exp(-.5 Ln)
                -> PE broadcast -> 3 DVE passes per chunk into out_r (f16).
                The mean-add passes are emitted first (they only need mb, so
                DVE runs them while the rstd ACT chain finishes). If
                `scratch` is given the passes write there, leaving x_master
                raw. Returns (mb, rb)."""
                def vslot(j, c):
                    return vec_t[:, j * HC + c: j * HC + c + 1]

                mneg_h = pst.tile([1, 512], F16, tag="stat16", bufs=4,
                                  name=f"mnegh_{tag}")
                nc.vector.tensor_scalar(mneg_h[:], sum_row, -1.0 / H, None,
                                        ALU.mult)
                m2 = pst.tile([1, 512], F32, tag="stat32", bufs=6,
                              name=f"m2_{tag}")
                nc.scalar.activation(m2[:], sum_row, AFT.Square,
                                     scale=1.0 / H)
                var = pst.tile([1, 512], F32, tag="stat32", bufs=6,
                               name=f"var_{tag}")
                nc.vector.scalar_tensor_tensor(var[:], sq_row, 1.0 / H,
                                               m2[:], ALU.mult, ALU.subtract)
                lnv = pst.tile([1, 512], F32, tag="stat32", bufs=6,
                               name=f"lnv_{tag}")
                nc.scalar.activation(lnv[:], var[:], AFT.Ln,
                                     bias=eps_t[0:1, :])
                rstd_h = pst.tile([1, 512], F16, tag="stat16", bufs=4,
                                  name=f"rstdh_{tag}")
                nc.scalar.activation(rstd_h[:], lnv[:], AFT.Exp, scale=-0.5)
                mb = psm.tile([128, 512], F16, tag="bcast", bufs=5,
                              name=f"mb_{tag}")
                pbcast(mb[:], mneg_h[0:1, :], bc_pool, bc_tag, bc_bufs,
                       f"mb_{tag}")
                tgt = scratch if scratch is not None else x_master
                for c in range(HC):
                    nc.vector.tensor_tensor(mmslice(tgt, c),
                                            mmslice(x_master, c), mb[:],
                                            ALU.add)

                def tail():
                    """rb broadcast (one PE matmul) + mult/affine passes.
                    Deferred by LN2 into the next layer's pipeline so PE has
                    chains to run while the rstd ACT chain finishes."""
                    rb = psm.tile([128, 512], F16, tag="bcast", bufs=5,
                                  name=f"rb_{tag}")
                    pbcast(rb[:], rstd_h[0:1, :], bc_pool, bc_tag, bc_bufs,
                           f"rb_{tag}")
                    for c in range(HC):
                        nc.vector.tensor_tensor(mmslice(tgt, c),
                                                mmslice(tgt, c), rb[:],
                                                ALU.mult)
                        nc.vector.tensor_scalar(mmslice(out_r, c),
                                                mmslice(tgt, c),
                                                vslot(g_j, c), vslot(b_j, c),
                                                ALU.mult, ALU.add)
                    return rb

                return mb, tail

            fpx = xT          # raw pre-LN2 master feeding Q/K (input at li=0)
            rb_prev = None    # LN2 rstd broadcast from the previous layer
            tneg_prev = None  # broadcast of -mu*rstd from the previous layer
            pending_ln2 = None  # deferred LN2 tail (rb bcast + mult/affine)
            for li in range(L):
                vec_t = pb.tile([128, 10 * HC], F32, tag="vec",
                                name=f"vec_{li}")
                nc.sync.dma_start(vec_t[:], vecs[li])
                bi_t = pb.tile([128, FC], F32, tag="biv", name=f"biv_{li}")
                nc.sync.dma_start(bi_t[:], biv[li])
                s1_t = pb.tile([128, FC], F32, tag="s1v", name=f"s1v_{li}")
                nc.sync.dma_start(s1_t[:], s1vt[li])

                def vslot(j, c):
                    return vec_t[:, j * HC + c: j * HC + c + 1]

                wq_t = [pw.tile([128, H], F16, tag="w768",
                                name=f"wq_{li}_{c}") for c in range(HC)]
                for c in range(HC):
                    nc.sync.dma_start(wq_t[c][:], Wq[li, c * 128:(c + 1) * 128, :])
                wk_t = [pw.tile([128, H], F16, tag="w768",
                                name=f"wk_{li}_{c}") for c in range(HC)]
                for c in range(HC):
                    nc.sync.dma_start(wk_t[c][:], Wk[li, c * 128:(c + 1) * 128, :])
                wv_t = [pw.tile([128, H], F16, tag="w768",
                                name=f"wv_{li}_{c}") for c in range(HC)]
                for c in range(HC):
                    nc.sync.dma_start(wv_t[c][:], Wv[li, c * 128:(c + 1) * 128, :])

                wo_t = [pw.tile([128, H], F16, tag="w768",
                                name=f"wo_{li}_{c}") for c in range(HC)]
                for c in range(HC):
                    nc.sync.dma_start(wo_t[c][:], Wo[li, c * 128:(c + 1) * 128, :])

                # ---- fused QKV + attention, software-pipelined over head
                # pairs. PE emission interleaves scores matmuls between the
                # Q/K/V accumulation chains so TensorE never waits on the
                # ACT exp stream (HAM stays warm); ctx runs one slot behind
                # scores, epilogue one behind ctx. PSUM budget: qkv 2 +
                # scores 2 + ctx 2 (+ global ps 2) = 8 banks.
                att_pool_cm = tc.tile_pool(name=f"att{li}", bufs=1,
                                           space="PSUM")
                pat = att_pool_cm.__enter__()

                pr_eo = {}
                ctx_eo = {}

                def emit_qkchain(m, w_t, dst, bias_j):
                    """Q/K chain on the RAW pre-LN2 master (fpx), evacuated
                    via ACT; the per-token LN fixup runs on DVE afterwards,
                    off the PE critical path (emit_qkfix)."""
                    q_ps = pat.tile([128, 512], F32, tag="qkp", bufs=2,
                                    name=f"{'qps' if bias_j == 0 else 'kps'}"
                                         f"_{li}_{m}")
                    for c in range(HC):
                        nc.tensor.matmul(q_ps[:],
                                         w_t[c][:, m * 128:(m + 1) * 128],
                                         mmslice(fpx, c), start=(c == 0),
                                         stop=(c == HC - 1),
                                         skip_group_check=True)
                    if li == 0:
                        nc.scalar.activation(mmslice(dst, m), q_ps[:],
                                             AFT.Identity, bias=vslot(bias_j, m))
                    else:
                        nc.scalar.activation(mmslice(dst, m), q_ps[:],
                                             AFT.Identity)

                def emit_qkfix(m, dst, bias_j, s_j):
                    """q = rstd*qraw - (mu*rstd)*S + b' (per-token LN2 fold)."""
                    if li == 0:
                        return
                    nc.vector.tensor_tensor(mmslice(dst, m),
                                            mmslice(dst, m), rb_prev[:],
                                            ALU.mult)
                    nc.vector.scalar_tensor_tensor(
                        mmslice(dst, m), tneg_prev[:], vslot(s_j, m),
                        mmslice(dst, m), ALU.mult, ALU.add)
                    nc.vector.tensor_scalar(mmslice(dst, m),
                                            mmslice(dst, m),
                                            vslot(bias_j, m), None,
                                            ALU.add)

                def emit_qchain(m):
                    emit_qkchain(m, wq_t, qT, 0)

                def emit_kchain(m):
                    emit_qkchain(m, wk_t, kT, 1)

                def emit_vgroup(sc, half):
                    v_ps = pat.tile([128, 384], F32, tag="qkp", bufs=2,
                                    name=f"vps_{li}_{sc}_{half}")
                    for c in range(HC):
                        nc.tensor.matmul(
                            v_ps[:],
                            xT[:, c * 512 + sc * 128:
                               c * 512 + (sc + 1) * 128],
                            wv_t[c][:, half * 384:(half + 1) * 384],
                            start=(c == 0), stop=(c == HC - 1),
                            skip_group_check=True)
                    v3 = v_ps[:].rearrange("p (h x c) -> p h x c", h=3, x=2)
                    nc.vector.tensor_copy(
                        vt4[:, sc, half * 6 + 0:half * 6 + 6:2, 0:64],
                        v3[:, :, 0, :])
                    nc.vector.tensor_copy(
                        vt4[:, sc, half * 6 + 1:half * 6 + 6:2, 64:128],
                        v3[:, :, 1, :])

                def emit_scores_kc(hp, kc):
                    c = hp
                    if kc == 0:
                        pr_eo[hp] = [ppr.tile([128, SC * 512], F16,
                                              tag="probs",
                                              name=f"probs_{li}_{2*hp+r}")
                                     for r in range(2)]
                    st_eo = [pat.tile([128, 512], F32, tag="satt", bufs=2,
                                      name=f"sps_{li}_{hp}_{r}_{kc}")
                             for r in range(2)]
                    # r0/r1 matmuls interleaved: disjoint PE row groups
                    # (rows 0-63 vs 64-127) execute concurrently
                    for r in range(2):
                        o = r * 64
                        nc.tensor.matmul(
                            st_eo[r][:],
                            kT[o:o + 64, c * 512 + kc * 128:
                               c * 512 + (kc + 1) * 128],
                            qT[o:o + 64, c * 512:(c + 1) * 512],
                            start=True, stop=True, skip_group_check=True)
                    for r in range(2):
                        nc.scalar.activation(
                            pr_eo[hp][r][:, kc * 512:(kc + 1) * 512],
                            st_eo[r][:], AFT.Exp,
                            scale=float(1.0 / np.sqrt(DH)))

                def emit_ctx_kc(hp, kcs):
                    for r in range(2):
                        h = 2 * hp + r
                        if (hp, r) not in ctx_eo:
                            ctx_eo[(hp, r)] = pat.tile(
                                [128, 512], F32, tag="ctx", bufs=2,
                                name=f"cps_{li}_{h}")
                        ctx_ps = ctx_eo[(hp, r)]
                        for kc in kcs:
                            lhs = (vt4[:, kc, h, 0:65] if r == 0
                                   else vt4[:, kc, h, 0:128])
                            nc.tensor.matmul(
                                ctx_ps[0:(65 if r == 0 else 128), :], lhs,
                                pr_eo[hp][r][:, kc * 512:(kc + 1) * 512],
                                start=(kc == 0), stop=(kc == SC - 1),
                                skip_group_check=True)

                def emit_epi(hp):
                    c = hp
                    c0 = ctx_eo.pop((hp, 0))
                    c1 = ctx_eo.pop((hp, 1))
                    del pr_eo[hp]
                    # softmax denominators sit in ctx PSUM rows (64 for even
                    # head, 0 for odd). 1/den via one fast-recip DVE op each
                    # (custom-DVE ops need base partition 0, so slice from
                    # row 0); PE K=1 matmul broadcasts the f32 rec row (as
                    # f32r moving) across partitions; one multiply per head
                    # normalizes ctx into ctxT.
                    from concourse.dve_ops import (RECIP_APPROX_FAST_CONSTS,
                                                   RECIPROCAL_APPROX_FAST)
                    rc = RECIP_APPROX_FAST_CONSTS
                    rec0 = psm.tile([128, 512], F16, tag="rec", bufs=2,
                                    name=f"reca_{li}_{hp}")
                    nc.vector._custom_dve(
                        RECIPROCAL_APPROX_FAST,
                        out=rec0[0:65, :], in0=c0[0:65, :],
                        s0=rc["s0"], s1=rc["s1"], imm2=rc["imm2"])
                    rec1 = psm.tile([128, 512], F16, tag="rec", bufs=2,
                                    name=f"recb_{li}_{hp}")
                    nc.vector._custom_dve(
                        RECIPROCAL_APPROX_FAST,
                        out=rec1[0:1, :], in0=c1[0:1, :],
                        s0=rc["s0"], s1=rc["s1"], imm2=rc["imm2"])
                    bb = psm.tile([128, 512], F16, tag="bsb", bufs=3,
                                  name=f"bb_{li}_{hp}")
                    bp0 = ps.tile([128, 512], F32, tag="ps", bufs=2,
                                  name=f"bpa_{li}_{hp}")
                    nc.tensor.matmul(bp0[:], ones_h[64:65, :],
                                     rec0[64:65, :], start=True, stop=True)
                    nc.vector.tensor_copy(bb[0:64, :], bp0[0:64, :])
                    bp1 = ps.tile([128, 512], F32, tag="ps", bufs=2,
                                  name=f"bpb_{li}_{hp}")
                    nc.tensor.matmul(bp1[:], ones_h[0:1, :],
                                     rec1[0:1, :], start=True, stop=True)
                    nc.vector.tensor_copy(bb[64:128, :], bp1[64:128, :])
                    nc.vector.tensor_tensor(
                        ctxT[0:64, c * 512:(c + 1) * 512],
                        c0[0:64, :], bb[0:64, :], ALU.mult)
                    nc.vector.tensor_tensor(
                        ctxT[64:128, c * 512:(c + 1) * 512],
                        c1[64:128, :], bb[64:128, :], ALU.mult)

                # pipeline: scores(hp) in slot hp+1 interleaved between the
                # Q/K/V chains (each chain gives ACT time to drain the
                # previous kc's exps); ctx(hp) in slot hp+2. V groups read
                # the NORMALIZED xT (ready a few us into the layer), so they
                # sit in slots 2-3, just before the ctx stages that consume
                # them (ctx(hp<3) needs half 0, ctx(hp>=3) half 1).
                vplan = {2: [(0, 0), (1, 0), (2, 0), (3, 0)],
                         3: [(0, 1), (1, 1), (2, 1), (3, 1)]}
                for slot in range(8):
                    hp_s = slot - 1
                    hp_c = slot - 2
                    vg = vplan.get(slot, [])
                    if 0 <= hp_s < 6:
                        emit_scores_kc(hp_s, 0)
                    if slot < HC:
                        emit_qchain(slot)
                    if 0 <= hp_s < 6:
                        emit_scores_kc(hp_s, 1)
                    if slot < HC:
                        emit_kchain(slot)
                    if slot == 0 and pending_ln2 is not None:
                        # previous layer's deferred LN2 tail: its rb
                        # broadcast matmul now sits behind this slot's 12
                        # chain matmuls, so PE stays busy while the rstd
                        # ACT chain completes.
                        mb2p, tail2p = pending_ln2
                        rb_prev = tail2p()
                        tneg_prev = psm.tile([128, 512], F16, tag="tneg",
                                             bufs=2, name=f"tneg_{li}")
                        nc.vector.tensor_tensor(tneg_prev[:], mb2p[:],
                                                rb_prev[:], ALU.mult)
                        pending_ln2 = None
                    if slot < HC:
                        emit_qkfix(slot, qT, 0, 8)
                        emit_qkfix(slot, kT, 1, 9)
                    if 0 <= hp_s < 6:
                        emit_scores_kc(hp_s, 2)
                    for v in vg[:2]:
                        emit_vgroup(*v)
                    if 0 <= hp_s < 6:
                        emit_scores_kc(hp_s, 3)
                    for v in vg[2:]:
                        emit_vgroup(*v)
                    if hp_c >= 0:
                        emit_ctx_kc(hp_c, (0, 1))
                        emit_ctx_kc(hp_c, (2, 3))
                        emit_epi(hp_c)

                att_pool_cm.__exit__(None, None, None)

                # ---- attn output projection + residual + inline LN1 stats --
                ln1_pool_cm = tc.tile_pool(name=f"lnp{li}", bufs=2,
                                           space="PSUM")
                pln = ln1_pool_cm.__enter__()
                ap_ = pf.tile([128, HC * 512], F16, tag="actf",
                              name=f"ap_{li}")
                l1sq = pr.tile([128, HC * 512], F16, tag="actr",
                               name=f"l1sq_{li}")
                sum1 = pln.tile([1, 512], F32, tag="lnp", name=f"sum1_{li}")
                sq1 = pln.tile([1, 512], F32, tag="lnp", name=f"sq1_{li}")
                for m in range(HC):
                    a_ps = ps.tile([128, 512], F32, tag="ps",
                                   name=f"aps_{li}_{m}")
                    # residual seeded via identity matmul; evacuation is a
                    # pure ACT Identity+bias (keeps DVE off the tail)
                    nc.tensor.matmul(a_ps[:], id_t[:], mmslice(xT, m),
                                     start=True, stop=False,
                                     skip_group_check=True)
                    for c in range(HC):
                        nc.tensor.matmul(a_ps[:],
                                         wo_t[c][:, m * 128:(m + 1) * 128],
                                         mmslice(ctxT, c), start=False,
                                         stop=(c == HC - 1),
                                         skip_group_check=True)
                    nc.scalar.activation(mmslice(ap_, m), a_ps[:],
                                         AFT.Identity, bias=vslot(2, m))
                    nc.vector.tensor_tensor(mmslice(l1sq, m),
                                            mmslice(ap_, m), mmslice(ap_, m),
                                            ALU.mult)
                    nc.tensor.matmul(sum1[:], onec[:, :], mmslice(ap_, m),
                                     start=(m == 0), stop=(m == HC - 1),
                                     skip_group_check=True)
                    nc.tensor.matmul(sq1[:], onec[:, :], mmslice(l1sq, m),
                                     start=(m == 0), stop=(m == HC - 1),
                                     skip_group_check=True)

                lo_r = pr.tile([128, HC * 512], F16, tag="actr",
                               name=f"lor_{li}")
                mb1, ln1_tail = ln_finish(ap_, sum1[:], sq1[:], 3, 4, vec_t,
                                          f"l1_{li}", lo_r, ps, "ps", 2,
                                          scratch=l1sq)
                ln1_pool_cm.__exit__(None, None, None)
                # prefetch the gelu ACT table before the first real gelu
                nc.scalar.activation(dumt[0:1, 1:2], dumt[0:1, 0:1],
                                     AFT.Gelu)

                # ---- FFN: f1 chains on the RAW ap_ (LN1 folded into Wi';
                # per-chunk rstd/mu fixup on DVE before gelu), g-groups
                # software-pipelined so PE runs chains of group g while the
                # fixup+gelu of group g-1 streams on DVE/ACT. The LN1 tail
                # (rb bcast + lo_r passes) is deferred behind group 0's
                # chains. ----
                ffn_pool_cm = tc.tile_pool(name=f"ffn{li}", bufs=6,
                                           space="PSUM")
                pacc = ffn_pool_cm.__enter__()
                acc_ps = [pacc.tile([128, 512], F32, tag="ffacc",
                                    name=f"facc_{li}_{m}") for m in range(HC)]
                raw_t = {}

                def emit_f1_chain(g, fg, wig):
                    f = g * HC + fg
                    f1_ps = ps.tile([128, 512], F32, tag="ps",
                                    name=f"f1_{li}_{f}")
                    for c in range(HC):
                        nc.tensor.matmul(
                            f1_ps[:],
                            wig[c][:, fg * 128:(fg + 1) * 128],
                            mmslice(ap_, c), start=(c == 0),
                            stop=(c == HC - 1), skip_group_check=True)
                    raw = pit.tile([128, 512], F16, tag="irw", bufs=7,
                                   name=f"itr_{li}_{f}")
                    nc.scalar.activation(raw[:], f1_ps[:], AFT.Identity)
                    raw_t[f] = raw

                def emit_fix_gelu_facc(f):
                    raw = raw_t.pop(f)
                    nc.vector.tensor_tensor(raw[:], raw[:], rb1[:], ALU.mult)
                    nc.vector.scalar_tensor_tensor(
                        raw[:], tneg1[:], s1_t[:, f:f + 1], raw[:],
                        ALU.mult, ALU.add)
                    inter = pit.tile([128, 512], F16, tag="inter", bufs=3,
                                     name=f"it_{li}_{f}")
                    nc.scalar.activation(inter[:], raw[:], AFT.Gelu,
                                         bias=bi_t[:, f:f + 1])
                    wo2_t = pw.tile([128, H], F16, tag="w768",
                                    name=f"wo2_{li}_{f}")
                    nc.sync.dma_start(wo2_t[:],
                                      Wo2[li, f * 128:(f + 1) * 128, :])
                    for m in range(HC):
                        nc.tensor.matmul(
                            acc_ps[m][:], wo2_t[:, m * 128:(m + 1) * 128],
                            inter[:], start=False, stop=(f == FC - 1),
                            skip_group_check=True)

                for g in range(5):
                    if g < 4:
                        wig = [pw.tile([128, H], F16, tag="w768",
                                       name=f"wi_{li}_{g}_{c}")
                               for c in range(HC)]
                        for c in range(HC):
                            nc.sync.dma_start(
                                wig[c][:],
                                Wi[li, c * 128:(c + 1) * 128,
                                   g * 768:(g + 1) * 768])
                        for fg in range(HC):
                            emit_f1_chain(g, fg, wig)
                    if g == 0:
                        # deferred LN1 tail behind group 0's 36 chain MMs
                        rb1 = ln1_tail()
                        tneg1 = psm.tile([128, 512], F16, tag="tneg",
                                         bufs=2, name=f"tneg1_{li}")
                        nc.vector.tensor_tensor(tneg1[:], mb1[:], rb1[:],
                                                ALU.mult)
                        for m in range(HC):
                            # residual (lo_r) seeded via identity matmul
                            nc.tensor.matmul(acc_ps[m][:], id_t[:],
                                             mmslice(lo_r, m),
                                             start=True, stop=False,
                                             skip_group_check=True)
                    else:
                        for fg in range(HC):
                            emit_fix_gelu_facc((g - 1) * HC + fg)
                # prefetch the ln/exp ACT table before LN2's Ln
                nc.scalar.activation(dumt[0:1, 1:2], dumt[0:1, 0:1], AFT.Ln)

                fp_ = pf.tile([128, HC * 512], F16, tag="actf",
                              name=f"fp_{li}")
                l2sq = pr.tile([128, HC * 512], F16, tag="actr",
                               name=f"l2sq_{li}")
                sum2 = ps.tile([1, 512], F32, tag="ps", name=f"sum2_{li}")
                sq2 = ps.tile([1, 512], F32, tag="ps", name=f"sq2_{li}")
                for m in range(HC):
                    nc.scalar.activation(mmslice(fp_, m), acc_ps[m][:],
                                         AFT.Identity, bias=vslot(7, m))
                    nc.vector.tensor_tensor(mmslice(l2sq, m),
                                            mmslice(fp_, m), mmslice(fp_, m),
                                            ALU.mult)
                    nc.tensor.matmul(sum2[:], onec[:, :], mmslice(fp_, m),
                                     start=(m == 0), stop=(m == HC - 1),
                                     skip_group_check=True)
                    nc.tensor.matmul(sq2[:], onec[:, :], mmslice(l2sq, m),
                                     start=(m == 0), stop=(m == HC - 1),
                                     skip_group_check=True)
                pending_ln2 = ln_finish(fp_, sum2[:], sq2[:], 5, 6, vec_t,
                                        f"l2_{li}", xT, ps, "ps", 2,
                                        scratch=l2sq)
                fpx = fp_
                ffn_pool_cm.__exit__(None, None, None)

            # flush the last layer's deferred LN2 tail into xT
            pending_ln2[1]()
            nc.sync.dma_start(outT.rearrange("(c p) s -> p c s", p=128),
                              xT[:].rearrange("p (c s) -> p c s", c=HC))

    nc.compile()
    return nc


def _get_runner():
    if "runner" in _CACHE:
        return _CACHE["runner"]
    import jax
    from jax.sharding import Mesh, PartitionSpec
    from jax.experimental.shard_map import shard_map
    from concourse import mybir
    from concourse.bass2jax import (_bass_exec_p, install_neuronx_cc_hook,
                                    partition_id_tensor)

    install_neuronx_cc_hook()
    nc = _build_program()

    pname = nc.partition_id_tensor.name if nc.partition_id_tensor else None
    in_names, out_names, out_avals, zero_outs = [], [], [], []
    for alloc in nc.m.functions[0].allocations:
        if not isinstance(alloc, mybir.MemoryLocationSet):
            continue
        name = alloc.memorylocations[0].name
        if alloc.kind == "ExternalInput":
            if name == pname:
                continue
            in_names.append(name)
        elif alloc.kind == "ExternalOutput":
            out_names.append(name)
            shape = tuple(alloc.tensor_shape)
            dtype = mybir.dt.np(alloc.dtype)
            out_avals.append(jax.core.ShapedArray(shape, dtype))
            zero_outs.append(np.zeros(shape, dtype))
    n_params = len(in_names)
    n_outs = len(out_avals)
    all_in_names = list(in_names) + list(out_names)
    if pname is not None:
        all_in_names = all_in_names + [pname]

    def _body(*args):
        operands = list(args)
        if pname is not None:
            operands.append(partition_id_tensor())
        outs = _bass_exec_p.bind(
            *operands,
            out_avals=tuple(out_avals),
            in_names=tuple(all_in_names),
            out_names=tuple(out_names),
            lowering_input_output_aliases=(),
            sim_require_finite=False,
            sim_require_nnan=False,
            nc=nc,
        )
        return tuple(outs)

    devices = jax.devices()[:B]
    mesh = Mesh(np.asarray(devices), ("core",))
    in_specs = (PartitionSpec("core"),) * (n_params + n_outs)
    out_specs = (PartitionSpec("core"),) * n_outs
    donate = tuple(range(n_params, n_params + n_outs))
    jitted = jax.jit(
        shard_map(_body, mesh=mesh, in_specs=in_specs, out_specs=out_specs,
                  check_rep=False),
        donate_argnums=donate, keep_unused=True)

    runner = {
        "jit": jitted, "in_names": in_names, "out_names": out_names,
        "zero_outs": zero_outs, "mesh": mesh, "devices": devices,
    }
    _CACHE["runner"] = runner
    return runner


def _prep_core_inputs(inputs):
    if L != 12:  # debug: KERNEL_LAYERS override slices the stacks
        inputs = {k: (v[:L] if k not in ("hidden_states", "attention_mask")
                      else v) for k, v in inputs.items()}
    hs = np.asarray(inputs["hidden_states"], np.float32)
    mask = np.asarray(inputs["attention_mask"], np.float32)
    if np.any(mask):
        raise NotImplementedError(
            "kernel compiled for the zero attention_mask this problem "
            "guarantees (spec fill=zeros); nonzero mask unsupported")
    Wv = np.asarray(inputs["Wv"], np.float16)
    Wo = np.asarray(inputs["Wo"], np.float16)
    Wo2 = np.asarray(inputs["Wo2"], np.float16)
    bq = np.asarray(inputs["bq"], np.float64)
    bk = np.asarray(inputs["bk"], np.float64)
    bv = np.asarray(inputs["bv"], np.float32)
    bo = np.asarray(inputs["bo"], np.float32)
    bi = np.asarray(inputs["bi"], np.float32)
    bo2 = np.asarray(inputs["bo2"], np.float32)
    g1 = np.asarray(inputs["ln1_g"], np.float32)
    b1 = np.asarray(inputs["ln1_b"], np.float32)
    g2 = np.asarray(inputs["ln2_g"], np.float32)
    b2 = np.asarray(inputs["ln2_b"], np.float32)

    # fold bv into bo: (ctx + bv) @ Wo + bo == ctx @ Wo + (bo + bv @ Wo)
    bo_eff = (bo.astype(np.float64)
              + np.einsum("lh,lho->lo", bv.astype(np.float64),
                          np.asarray(inputs["Wo"], np.float64))
              ).astype(np.float32)

    # fold the previous layer's LN2 affine (g2, b2) into Q/K:
    #   q = Wq'^T·nrm + bq',  Wq' = g2_prev ⊙rows Wq,  bq' = bq + b2_prev@Wq
    # where nrm = (x_raw − μ)·rstd; the kernel computes Wq'^T·x_raw and
    # fixes up with  rstd·qraw − (μ·rstd)·Sq + bq'  (Sq = colsum(Wq')).
    Wq64 = np.asarray(inputs["Wq"], np.float64).copy()
    Wk64 = np.asarray(inputs["Wk"], np.float64).copy()
    Sq = np.zeros((L, H), np.float64)
    Sk = np.zeros((L, H), np.float64)
    bq_eff = bq.copy()
    bk_eff = bk.copy()
    for li in range(1, L):
        g2p = g2[li - 1].astype(np.float64)
        b2p = b2[li - 1].astype(np.float64)
        bq_eff[li] = bq[li] + b2p @ Wq64[li]
        bk_eff[li] = bk[li] + b2p @ Wk64[li]
        Wq64[li] *= g2p[:, None]
        Wk64[li] *= g2p[:, None]
        Sq[li] = Wq64[li].sum(axis=0)
        Sk[li] = Wk64[li].sum(axis=0)
    Wq = Wq64.astype(np.float16)
    Wk = Wk64.astype(np.float16)
    # Sq/Sk must match the f16 weights actually used by the matmuls
    Sq = Wq.astype(np.float64).sum(axis=1)
    Sk = Wk.astype(np.float64).sum(axis=1)
    Sq[0] = 0.0
    Sk[0] = 0.0

    # fold LN1's affine (g1, b1) into FFN1 the same way:
    #   inter_pre = Wi'^T·nrm1 + bi',  Wi' = g1 ⊙rows Wi,  bi' = bi + b1@Wi
    # kernel computes Wi'^T·ap_raw and fixes up with rstd1/mu1 and
    # S1 = colsum(Wi').
    Wi64 = np.asarray(inputs["Wi"], np.float64).copy()
    bi_eff = np.asarray(inputs["bi"], np.float64).copy()
    for li in range(L):
        g1p = g1[li].astype(np.float64)
        b1p = b1[li].astype(np.float64)
        bi_eff[li] = bi_eff[li] + b1p @ Wi64[li]
        Wi64[li] *= g1p[:, None]
    Wi = Wi64.astype(np.float16)
    S1 = Wi.astype(np.float64).sum(axis=1)  # [L, FF] colsums of f16 Wi'
    bi = bi_eff.astype(np.float32)

    def pack768(v):  # [L,768] -> [L,128,HC] with [l,p,c] = v[l, c*128+p]
        v = np.asarray(v, np.float32)
        return np.ascontiguousarray(v.reshape(L, HC, 128).transpose(0, 2, 1))

    vecs = np.stack([pack768(v) for v in
                     (bq_eff, bk_eff, bo_eff, g1, b1, g2, b2, bo2, Sq, Sk)],
                    axis=2)
    # [L,128,10,HC] -> [L,128,10*HC]
    vecs = np.ascontiguousarray(vecs.reshape(L, 128, 10 * HC))
    biv = np.ascontiguousarray(bi.reshape(L, FC, 128).transpose(0, 2, 1))
    s1v = np.ascontiguousarray(
        S1.astype(np.float32).reshape(L, FC, 128).transpose(0, 2, 1))

    per_core = {
        "hsT": [np.ascontiguousarray(hs[b].T.astype(np.float16))
                for b in range(B)],
        "ident": [np.eye(128, dtype=np.float16)] * B,
    }
    for name, arr in (("Wq", Wq), ("Wk", Wk), ("Wv", Wv), ("Wo", Wo),
                      ("Wi", Wi), ("Wo2", Wo2), ("vecs", vecs), ("biv", biv),
                      ("s1v", s1v)):
        per_core[name] = [arr] * B
    return per_core


def run_on_device(inputs, n_timing_runs=0):
    """Execute; returns (output [B,S,H] fp32, exec_seconds or None)."""
    import jax
    from jax.sharding import NamedSharding, PartitionSpec
    runner = _get_runner()
    per_core = _prep_core_inputs(inputs)
    devices = runner["devices"]
    mesh = runner["mesh"]
    sharding = NamedSharding(mesh, PartitionSpec("core"))

    global_args = []
    for name in runner["in_names"]:
        shards = per_core[name]
        arrs = [jax.device_put(shards[c], devices[c]) for c in range(B)]
        gshape = (B * shards[0].shape[0],) + shards[0].shape[1:]
        global_args.append(
            jax.make_array_from_single_device_arrays(gshape, sharding, arrs))

    def zeros_args():
        outs = []
        for z in runner["zero_outs"]:
            arrs = [jax.device_put(z, devices[c]) for c in range(B)]
            gshape = (B * z.shape[0],) + z.shape[1:]
            outs.append(jax.make_array_from_single_device_arrays(
                gshape, sharding, arrs))
        return outs

    out_arrs = runner["jit"](*global_args, *zeros_args())
    jax.block_until_ready(out_arrs)

    exec_s = None
    if n_timing_runs > 0:
        import time
        times = []
        for _ in range(n_timing_runs):
            zo = zeros_args()
            jax.block_until_ready(zo)
            t0 = time.perf_counter()
            out_arrs = runner["jit"](*global_args, *zo)
            jax.block_until_ready(out_arrs)
            times.append(time.perf_counter() - t0)
        exec_s = min(times)

    outT = np.asarray(out_arrs[0]).astype(np.float32).reshape(B, H, S)
    out = np.ascontiguousarray(outT.transpose(0, 2, 1))
    return out, exec_s


def kernel(**inputs) -> np.ndarray:
    out, _ = run_on_device(inputs, n_timing_runs=0)
    return out

